# revision 33
# baseline (speedup 1.0000x reference)
"""HGT link predictor on 8 trn2 NeuronCores.

Sharding: nodes split 8 ways per type (2500/core, padded to 2560).
Params replicated. Edges partitioned by destination core, sorted by dst,
packed into 128-edge chunks within 128-dst-node windows.

v5 design (on top of v2):
- All matmul operands bf16 (PSUM accumulates fp32); a_rel folded into
  per-relation q projections; m_rel applied post-aggregation; scatter-add
  via one-hot matmuls with the softmax denominator riding as 8 extra
  columns (as in v2).
- Layer 0 needs NO collectives: x is replicated on every core, so each
  core computes the full-N layer-0 k/v table locally. The input
  projection for the kv path is done directly in feature-major form
  (hT = relu(Win^T @ x^T), two 128-wide matmuls per node tile -- no
  transposes), then k/v projections write the full 20480-row kv table
  straight to local DRAM. Only layer 1 performs the two kv AllGathers.
- Collectives block the issuing (gpsimd) engine queue until completion,
  and dma_gather lives on the same queue. The schedule orders Pool work
  so every gather that follows an AllGather in queue order also depends
  on it: [all layer-0 gathers] AG1(t0) [r0' gathers] AG1(t1) [r2'/r1'/
  r3' gathers]. kv projections for layer 1 are produced inside layer
  0's per-type finish stages so each AllGather's input is staged well
  before the queue reaches it.
- Deferred per-dst-type phase 2: relation processing only accumulates the
  normalized, m_rel-transformed aggregate (feature-major bf16). gelu +
  Wa + gated skip + residual + LayerNorm run as one batched pass per dst
  type, cutting ACT function-table swaps (exp vs gelu sets), and in the
  last layer each type flows straight into the output projection.
- Edges are packed into 128-edge chunks within 256-dst-node windows
  (84% slot utilization vs 59% at 128-node windows), shrinking gather
  rows and per-edge DVE work by ~30%; the one-hot scatter runs as two
  128-wide matmuls per chunk into the two PSUM window halves.
- Engine-flexible copies/relus use nc.any so the Tile scheduler routes
  them to whichever of DVE/ACT is idle.
"""
import math
import numpy as np

import concourse.bacc as bacc
import concourse.bass as bass
import concourse.mybir as mybir
import concourse.tile as tile
from concourse.bass_utils import run_bass_kernel_spmd
from concourse.library_config import mlp

F32 = mybir.dt.float32
BF16 = mybir.dt.bfloat16
I16 = mybir.dt.int16
AF = mybir.ActivationFunctionType
OP = mybir.AluOpType

T, R, L = 3, 4, 2
H, HEADS, D, FIN, OUT = 256, 8, 32, 128, 128
SRC_T = (0, 1, 1, 1)
DST_T = (1, 0, 1, 2)
LN_EPS = 1e-5
NC = 8
N = 20000
NL = N // NC          # 2500 real local nodes per type
NT = 20               # node tiles of 128
NLP = NT * 128        # 2560 padded local nodes
NWIN = NT             # dst windows of 128 local nodes
W2 = 256              # edge-packing window: 256 dst nodes (2 psum windows)
NWIN2 = NLP // W2     # 10 packing windows
KV_W = 2 * H          # 512: [k || v] columns of a kv-table row

# edge-relation processing order per layer (see module docstring)
EDGE_ORDERS = ((1, 2, 0, 3), (0, 2, 1, 3))


def _block_diag(a):
    """a: [HEADS, D, D] -> [H, H] block diagonal."""
    out = np.zeros((H, H), np.float32)
    for h in range(HEADS):
        out[h * D:(h + 1) * D, h * D:(h + 1) * D] = a[h]
    return out


def _wrap_idx(idx):
    """idx [M] -> [128, M//16] int16 wrapped in 16 partitions, replicated."""
    m = idx.shape[0]
    assert m % 16 == 0
    w = np.zeros((128, m // 16), np.int16)
    w[:16] = idx.astype(np.int16).reshape(m // 16, 16).T
    for rep in range(1, 8):
        w[16 * rep:16 * rep + 16] = w[:16]
    return w


def _preprocess(inputs):
    x = np.asarray(inputs["x"], np.float32)
    edge_index = np.asarray(inputs["edge_index"])
    Win = np.asarray(inputs["Win"], np.float32)
    b_in = np.asarray(inputs["b_in"], np.float32)
    Wk = np.asarray(inputs["Wk"], np.float32); bk = np.asarray(inputs["bk"], np.float32)
    Wq = np.asarray(inputs["Wq"], np.float32); bq = np.asarray(inputs["bq"], np.float32)
    Wv = np.asarray(inputs["Wv"], np.float32); bv = np.asarray(inputs["bv"], np.float32)
    Wa = np.asarray(inputs["Wa"], np.float32); ba = np.asarray(inputs["ba"], np.float32)
    skip = np.asarray(inputs["skip"], np.float32)
    a_rel = np.asarray(inputs["a_rel"], np.float32)
    m_rel = np.asarray(inputs["m_rel"], np.float32)
    p_rel = np.asarray(inputs["p_rel"], np.float32)
    ln_g = np.asarray(inputs["ln_g"], np.float32)
    ln_b = np.asarray(inputs["ln_b"], np.float32)
    Wout = np.asarray(inputs["Wout"], np.float32)
    bout = np.asarray(inputs["bout"], np.float32)

    meta = {}
    inv_sqrt_d = 1.0 / math.sqrt(D)
    # fold a_rel (scaled) into dst-side q projections per relation
    wq_eff = np.zeros((L, R, H, H), np.float32)
    bq_eff = np.zeros((L, R, H), np.float32)
    # block-diag m_rel chunks for post-aggregation transform (lhsT layout)
    m_blk = np.zeros((L, R, 2, 128, 128), np.float32)
    for l in range(L):
        for r in range(R):
            dt = DST_T[r]
            at = _block_diag(np.transpose(a_rel[l, r], (0, 2, 1))
                             * (p_rel[l, r] * inv_sqrt_d)[:, None, None])
            wq_eff[l, r] = Wq[l, dt] @ at
            bq_eff[l, r] = bq[l, dt] @ at
            mb = _block_diag(m_rel[l, r])
            m_blk[l, r, 0] = mb[0:128, 0:128]
            m_blk[l, r, 1] = mb[128:256, 128:256]
    beta = 1.0 / (1.0 + np.exp(-skip))          # [L, T]
    g = beta / (2.0 - beta)
    wa_eff = Wa * g[:, :, None, None]
    ba_eff = ba * g[:, :, None]
    meta["eps_eff"] = (LN_EPS / (2.0 - beta) ** 2).tolist()

    meta["use_bias"] = dict(
        bin_=bool(np.any(b_in)), bq=bool(np.any(bq_eff)),
        bkv=bool(np.any(bk[:, :2])) or bool(np.any(bv[:, :2])),
        ba=bool(np.any(ba_eff)), bout=bool(np.any(bout)),
        lng=not np.allclose(ln_g, 1.0), lnb=bool(np.any(ln_b)),
    )

    def bcast(v):
        # [..., F] -> [..., 128, F]: per-feature vectors replicated across partitions
        return np.ascontiguousarray(
            np.broadcast_to(v[..., None, :], v.shape[:-1] + (128, v.shape[-1])))

    # edge partitioning ---------------------------------------------------
    # edges are packed densely into 128-edge chunks within 256-dst-node
    # windows (better chunk utilization than 128-node windows); the one-hot
    # scatter runs as two 128-wide matmuls per chunk.
    win_edges = [[] for _ in range(NC)]   # [c][r][w2] -> (src_rows, dst_loc)
    kch_need = 1
    for c in range(NC):
        rel = []
        for r in range(R):
            src = edge_index[r, 0].astype(np.int64)
            dst = edge_index[r, 1].astype(np.int64)
            sel = (dst // NL) == c
            s, d = src[sel], dst[sel] - c * NL
            o = np.argsort(d, kind="stable")
            s, d = s[o], d[o]
            wins = []
            for w in range(NWIN2):
                m = (d // W2) == w
                sw, dw = s[m], d[m]
                kch_need = max(kch_need, (len(sw) + 127) // 128)
                wins.append((sw, dw))
            rel.append(wins)
        win_edges[c] = rel
    KCH = kch_need
    meta["KCH"] = KCH
    NCHUNK = NWIN2 * KCH
    NIDX_R = NCHUNK * 128

    per_core = []
    for c in range(NC):
        oh = np.zeros((R, NCHUNK, 128, W2), np.float32)
        kv_idx = np.zeros((R, NIDX_R), np.int64)
        qi_idx = np.zeros((R, NIDX_R), np.int64)
        for r in range(R):
            for w in range(NWIN2):
                sw, dw = win_edges[c][r][w]
                ne = len(sw)
                base = w * KCH * 128
                # src node n (global) -> kv-table row (n//NL)*NLP + n%NL
                kv_idx[r, base:base + ne] = (sw // NL) * NLP + (sw % NL)
                qi_idx[r, base:base + ne] = dw
                ch = base // 128 + np.arange(ne) // 128
                oh[r, ch, np.arange(ne) % 128, dw - w * W2] = 1.0
        # partition-major one-hot halves: [R, 128(edge), NCHUNK, 2, 128(col)]
        oh_pm = np.ascontiguousarray(
            oh.transpose(0, 2, 1, 3).reshape(R, 128, NCHUNK, 2, 128))
        xc = np.zeros((T, 128, NLP), np.float32)
        xc[:, :, :NL] = x[:, c * NL:(c + 1) * NL, :].transpose(0, 2, 1)
        per_core.append(dict(
            xT_h=_bf(xc),
            oh=_bf(oh_pm),
            kv_idx=np.stack([_wrap_idx(kv_idx[r]) for r in range(R)]),
            qi_idx=np.stack([_wrap_idx(qi_idx[r]) for r in range(R)]),
        ))

    # full-N x for source types 0/1, feature-major, padded per core shard
    # (row of global node n of type t in the kv table: (n//NL)*NLP + n%NL)
    xT_full = np.zeros((2, 128, NC * NLP), np.float32)
    for c in range(NC):
        xT_full[:, :, c * NLP:c * NLP + NL] = \
            x[:2, c * NL:(c + 1) * NL, :].transpose(0, 2, 1)

    shared = dict(
        xT_full=_bf(xT_full),
        win=_bf(Win),                                     # [3,128,256]
        wk=_bf(Wk[:, :2]), wv=_bf(Wv[:, :2]),             # [L,2,256,256]
        wq=_bf(wq_eff), wa=_bf(wa_eff),
        m_blk=_bf(m_blk),
        wout=_bf(Wout),
        ident=np.eye(128, dtype=np.float32),
        identb=_bf(np.eye(128, dtype=np.float32)),
        bin_b=bcast(b_in), bq_b=bcast(bq_eff),
        bk_b=bcast(bk[:, :2]), bv_b=bcast(bv[:, :2]),
        ba_b=bcast(ba_eff), bout_b=bcast(bout),
        lng_b=bcast(ln_g), lnb_b=bcast(ln_b),
    )
    return shared, per_core, meta


def _bf(a):
    import ml_dtypes
    return np.ascontiguousarray(a).astype(ml_dtypes.bfloat16)


def NIDX_R16(KCH):
    return NWIN2 * KCH * 128 // 16


def _build(nc, meta, shapes):
    KCH = meta["KCH"]
    NCHUNK = NWIN2 * KCH
    GC = KCH                             # chunks per gather group (1 window)
    NGRP = NWIN2
    ub = meta["use_bias"]
    eps_eff = meta["eps_eff"]

    def din(name, dt_):
        return nc.dram_tensor(name, shapes[name], dt_, kind="ExternalInput").ap()

    xT_h = din("xT_h", BF16); oh_d = din("oh", BF16)
    xT_full = din("xT_full", BF16)
    kv_idx_d = din("kv_idx", I16); qi_idx_d = din("qi_idx", I16)
    win_d = din("win", BF16)
    wk_d = din("wk", BF16); wv_d = din("wv", BF16)
    wq_d = din("wq", BF16); wa_d = din("wa", BF16)
    m_blk_d = din("m_blk", BF16)
    wout_d = din("wout", BF16)
    ident_d = din("ident", F32); identb_d = din("identb", BF16)
    bias_d = {k: din(k, F32) for k in
              ("bin_b", "bq_b", "bk_b", "bv_b", "ba_b", "bout_b", "lng_b", "lnb_b")}
    y_d = nc.dram_tensor("y", [T, NLP, OUT], F32, kind="ExternalOutput").ap()

    def bc32(ap2d):
        """[..., k] AP -> [..., k, 32] stride-0 broadcast AP."""
        return bass.AP(tensor=ap2d.tensor, offset=ap2d.offset,
                       ap=list(ap2d.ap) + [[0, D]])

    with tile.TileContext(nc) as tc:
        with (
            tc.tile_pool(name="persist", bufs=1) as pp,
            tc.tile_pool(name="wpool", bufs=6) as wp,
            tc.tile_pool(name="wsmall", bufs=2) as ws,
            tc.tile_pool(name="stage", bufs=2) as stg,
            tc.tile_pool(name="gathk", bufs=4) as gkp,
            tc.tile_pool(name="gath", bufs=3) as gep,
            tc.tile_pool(name="edge", bufs=2) as ep,
            tc.tile_pool(name="small", bufs=3) as sp,
            tc.tile_pool(name="gelu", bufs=1) as gp,
            tc.tile_pool(name="idx", bufs=2) as ip,
            tc.tile_pool(name="psSC", bufs=3, space="PSUM") as psSC,
            tc.tile_pool(name="psAG", bufs=2, space="PSUM") as psAG,
            tc.tile_pool(name="psPO", bufs=3, space="PSUM") as psPO,
            tc.tile_pool(name="dram", bufs=1, space="DRAM") as dp,
        ):
            nc.gpsimd.load_library(mlp)

            ident = pp.tile([128, 128], F32, tag="ident")
            nc.sync.dma_start(ident[:], ident_d)
            identb = pp.tile([128, 128], BF16, tag="identb")
            nc.sync.dma_start(identb[:], identb_d)
            h = pp.tile([128, T, NT, H], F32, tag="h")
            # feature-major normalized+m_rel aggregate, phase-2 input.
            # Slot 0 holds dst type 1 (accumulated across its two relations,
            # long-lived); types 0 and 2 take turns in slot 1.
            aggF = pp.tile([128, 2, 2, NT, 128], BF16, tag="aggF")
            AGG_SLOT = {1: 0, 0: 1, 2: 1}
            # feature-major h of types 0/1 (kv + q projection input)
            hTbig = pp.tile([128, 2, 2, NT, 128], BF16, tag="hTbig")
            wo_t = pp.tile([128, 2, OUT], BF16, tag="wo")
            nc.sync.dma_start(wo_t[:], wout_d.rearrange("(kt kp) m -> kp kt m", kp=128))

            # layer 0: full-N kv tables built locally (x is replicated);
            # layer 1: per-core shard staged to kv_loc then AllGathered.
            kv_loc = [None, [dp.tile([NLP, KV_W], BF16, name=f"kv_loc1{t}")
                             for t in range(2)]]
            kv_full = [[dp.tile([NC * NLP, KV_W], BF16, name=f"kv_full0{t}")
                        for t in range(2)],
                       [dp.tile([NC * NLP, KV_W], BF16, addr_space="Shared",
                                name=f"kv_full1{t}")
                        for t in range(2)]]
            q_dram = [dp.tile([R, NLP, H], BF16, name=f"q_dram{l}")
                      for l in range(L)]

            def load_w(src_ap):
                """[256, M] bf16 dram -> [128, 2, M] sbuf tile."""
                m = src_ap.shape[-1]
                t_ = wp.tile([128, 2, m], BF16, tag="w")
                nc.sync.dma_start(t_[:], src_ap.rearrange("(kt kp) m -> kp kt m", kp=128))
                return t_

            def load_bias(src_ap):
                t_ = wp.tile([128, H], F32, tag="bias")
                nc.sync.dma_start(t_[:], src_ap)
                return t_

            # ---- input projection: h[t] = relu(xT^T @ Win + b) ----
            def inproj_type(t):
                w_in = ws.tile([128, H], BF16, tag="win", name="w_in")
                nc.sync.dma_start(w_in[:], win_d[t])
                bt = load_bias(bias_d["bin_b"][t]) if ub["bin_"] else None
                for nt in range(NT):
                    xt = ws.tile([128, 128], BF16, tag="xt", name="xt")
                    nc.sync.dma_start(xt[:], xT_h[t, :, nt * 128:(nt + 1) * 128])
                    ps = psPO.tile([128, H], F32, tag="po", name="ps_in")
                    nc.tensor.matmul(ps[:], xt[:], w_in[:], start=True, stop=True)
                    if bt is not None:
                        nc.vector.tensor_add(ps[:], ps[:], bt[:])
                    nc.any.tensor_scalar(h[:, t, nt, :], ps[:], 0.0, None, OP.max)

            def transpose_tile(src2, nt_label):
                """h tile [128, 256] f32 -> hT [128, 2, 128] bf16 (feature-major)."""
                hTt = ws.tile([128, 2, 128], BF16, tag="hTt")
                for ft in range(2):
                    tp = psSC.tile([128, 128], F32, tag="sc")
                    nc.tensor.transpose(tp[:], src2[:, ft * 128:(ft + 1) * 128], ident[:])
                    nc.any.tensor_copy(hTt[:, ft, :], tp[:])
                return hTt

            def transpose_into_hTbig(t):
                """Refresh hTbig[t] from the current h[t]."""
                for nt in range(NT):
                    for ft in range(2):
                        tp = psSC.tile([128, 128], F32, tag="sc", name="tp")
                        nc.tensor.transpose(
                            tp[:], h[:, t, nt, ft * 128:(ft + 1) * 128], ident[:])
                        nc.any.tensor_copy(hTbig[:, t, ft, nt, :], tp[:])

            def fullN_kv_build(t):
                """Layer-0 kv table for src type t over ALL N nodes, built
                locally from the replicated x. The input projection runs
                directly in feature-major form (hT = relu(Win^T @ xT)), so
                no transposes are needed; k/v write straight to local DRAM."""
                w_in = ws.tile([128, H], BF16, tag="win", name="w_in0")
                nc.sync.dma_start(w_in[:], win_d[t])
                wkt = load_w(wk_d[0, t]); wvt = load_w(wv_d[0, t])
                GSTG = 5
                for grp in range(NC * NT // GSTG):
                    kvstg = stg.tile([128, GSTG, KV_W], BF16, tag="kvstg",
                                     name="kvstgF")
                    xt = ws.tile([128, GSTG, 128], BF16, tag="xt", name="xtF")
                    nc.sync.dma_start(
                        xt[:], xT_full[t, :, grp * GSTG * 128:(grp + 1) * GSTG * 128]
                        .rearrange("p (gi n) -> p gi n", gi=GSTG))
                    for gi in range(GSTG):
                        g = grp * GSTG + gi
                        ps = psAG.tile([128, 2, 128], F32, tag="ag", name="ps_hT")
                        for half in range(2):
                            nc.tensor.matmul(
                                ps[:, half, :],
                                w_in[:, half * 128:(half + 1) * 128], xt[:, gi, :],
                                start=True, stop=True)
                        hTn = ws.tile([128, 2, 128], BF16, tag="hTn", name="hTn")
                        nc.any.tensor_scalar(hTn[:], ps[:], 0.0, None, OP.max)
                        for (wt, col) in ((wkt, 0), (wvt, H)):
                            po = psPO.tile([128, H], F32, tag="po", name="ps_kvF")
                            for kt in range(2):
                                nc.tensor.matmul(po[:], hTn[:, kt, :],
                                                 wt[:, kt, :],
                                                 start=(kt == 0), stop=(kt == 1))
                            nc.any.tensor_copy(kvstg[:, gi, col:col + H], po[:])
                    r0_ = grp * GSTG * 128
                    r1_ = r0_ + GSTG * 128
                    nc.sync.dma_start(
                        kv_full[0][t][r0_:r1_, :].rearrange(
                            "(nt kp) m -> kp nt m", kp=128),
                        kvstg[:])

            def kvproj(l, t):
                """kv projections for layer l source type t from hTbig[t],
                staged to kv_loc (the AllGather trigger is emitted separately
                so it lands in the right spot in the Pool queue order)."""
                wkt = load_w(wk_d[l, t]); wvt = load_w(wv_d[l, t])
                bkt = load_bias(bias_d["bk_b"][l, t]) if ub["bkv"] else None
                bvt = load_bias(bias_d["bv_b"][l, t]) if ub["bkv"] else None
                QTR = NT // 4
                for quarter in range(4):
                    kvstg = stg.tile([128, QTR, KV_W], BF16, tag="kvstg",
                                     name="kvstg")
                    for nti in range(QTR):
                        nt = quarter * QTR + nti
                        for ci, (wt, bt, col) in enumerate(
                                ((wkt, bkt, 0), (wvt, bvt, H))):
                            ps = psPO.tile([128, H], F32, tag="po", name="ps_kv")
                            for kt in range(2):
                                nc.tensor.matmul(ps[:], hTbig[:, t, kt, nt, :],
                                                 wt[:, kt, :],
                                                 start=(kt == 0), stop=(kt == 1))
                            dst_ = kvstg[:, nti, col:col + H]
                            if bt is not None:
                                nc.vector.tensor_add(dst_, ps[:], bt[:])
                            else:
                                nc.any.tensor_copy(dst_, ps[:])
                    r0_ = quarter * QTR * 128
                    r1_ = r0_ + QTR * 128
                    nc.sync.dma_start(
                        kv_loc[l][t][r0_:r1_, :].rearrange(
                            "(nt kp) m -> kp nt m", kp=128),
                        kvstg[:])

            def ag_trigger(l, t, after=None):
                """AllGather kv_loc -> kv_full. Blocks the Pool queue until
                the collective completes, so call sites place this only
                where all following Pool work depends on it anyway.
                `after` pins the trigger behind a gather instruction so the
                scheduler cannot insert the collective mid-stream and hijack
                gathers the collective's own input depends on."""
                with nc.named_scope(f"l{l}_ag{t}"):
                    cc = nc.gpsimd.collective_compute(
                        "AllGather", OP.bypass,
                        replica_groups=[list(range(NC))],
                        ins=[kv_loc[l][t][:].opt()],
                        outs=[kv_full[l][t][:].opt()],
                    )
                    if after is not None:
                        bass._add_dep_helper(
                            cc.ins, after.ins, sync=False,
                            reason="AG waits for the gathers feeding its input")
                return cc

            def qproj(l, r):
                """q' projection for relation r (a_rel folded), to q_dram."""
                dt_ = DST_T[r]
                wqr = load_w(wq_d[l, r])
                bqr = load_bias(bias_d["bq_b"][l, r]) if ub["bq"] else None
                for quarter in range(4):
                    qstg = stg.tile([128, NT // 4, H], BF16, tag="qstg",
                                    name=f"qstg{r}")
                    for nti in range(NT // 4):
                        nt = quarter * (NT // 4) + nti
                        if dt_ < 2:
                            hTt2 = hTbig[:, dt_, :, nt, :]
                        else:
                            hTt = transpose_tile(h[:, dt_, nt, :], nt)
                            hTt2 = hTt[:]
                        ps = psPO.tile([128, H], F32, tag="po", name="ps_q")
                        for kt in range(2):
                            nc.tensor.matmul(ps[:], hTt2[:, kt, :],
                                             wqr[:, kt, :],
                                             start=(kt == 0), stop=(kt == 1))
                        if bqr is not None:
                            nc.vector.tensor_add(qstg[:, nti, :], ps[:], bqr[:])
                        else:
                            nc.any.tensor_copy(qstg[:, nti, :], ps[:])
                    r0_ = quarter * (NT // 4) * 128
                    r1_ = r0_ + (NT // 4) * 128
                    nc.sync.dma_start(
                        q_dram[l][r, r0_:r1_, :].rearrange(
                            "(nt kp) m -> kp nt m", kp=128),
                        qstg[:])

            def edge_rel(l, r, first_for_dst, after_cc=None, vw_pool=False):
                """Edge phase for relation r: gather kv/q', logits, softmax
                numerators, one-hot scatter, normalize, m_rel; accumulate
                feature-major result into aggF[dst]."""
                _sid, _ = nc.enter_named_scope(f"l{l}_r{r}", False)
                dt = DST_T[r]
                st = SRC_T[r]
                mblk_t = ws.tile([128, 2, 128], BF16, tag="mblk")
                nc.sync.dma_start(mblk_t[:], m_blk_d[l, r].rearrange("kt p m -> p kt m"))
                kvi = ip.tile([128, NIDX_R16(KCH)], I16, tag="kvi")
                qii = ip.tile([128, NIDX_R16(KCH)], I16, tag="qii")
                nc.sync.dma_start(kvi[:], kv_idx_d[r])
                nc.sync.dma_start(qii[:], qi_idx_d[r])
                for gidx in range(NGRP):
                    ni = GC * 128
                    kvg = gkp.tile([128, GC, KV_W], BF16, tag="kvg")
                    qig = gep.tile([128, GC, H], BF16, tag="qig")
                    gi_ = nc.gpsimd.dma_gather(
                        kvg[:], kv_full[l][st][:],
                        kvi[:, gidx * (ni // 16):(gidx + 1) * (ni // 16)],
                        ni, ni, KV_W)
                    if after_cc is not None and gidx == 0:
                        bass._add_dep_helper(
                            gi_.ins, after_cc.ins, sync=False,
                            reason="keep Pool queue clear ahead of the AG")
                    if gidx == NGRP - 1:
                        last_gather = gi_
                    nc.gpsimd.dma_gather(
                        qig[:], q_dram[l][r],
                        qii[:, gidx * (ni // 16):(gidx + 1) * (ni // 16)],
                        ni, ni, H)
                    ohg = ep.tile([128, GC, 2, 128], BF16, tag="ohg")
                    nc.sync.dma_start(ohg[:], oh_d[r, :, gidx * GC:(gidx + 1) * GC, :, :])
                    msg = ep.tile([128, GC, H + HEADS], BF16, tag="msg")
                    lg = sp.tile([128, GC, HEADS], F32, tag="lg")
                    # q*k product staged in msg[:, :, 0:H]; overwritten by
                    # the weighted-v below after the reduce consumes it.
                    # Runs on gpsimd: it depends only on the two gathers
                    # (same engine, just upstream), Pool sits idle between
                    # gathers, and DVE is the saturated engine.
                    nc.gpsimd.tensor_mul(msg[:, :, 0:H], qig[:], kvg[:, :, 0:H])
                    nc.vector.tensor_reduce(
                        lg[:], msg[:, :, 0:H].rearrange("p g (hh dd) -> p g hh dd", dd=D),
                        mybir.AxisListType.X, OP.add)
                    nc.scalar.activation(msg[:, :, H:H + HEADS], lg[:], AF.Exp)
                    # the last relation in the schedule may run this on
                    # gpsimd too (Pool has nothing queued after it)
                    vw_eng = nc.gpsimd if vw_pool else nc.vector
                    vw_eng.tensor_mul(
                        msg[:, :, 0:H].rearrange("p g (hh dd) -> p g hh dd", dd=D),
                        kvg[:, :, H:2 * H].rearrange("p g (hh dd) -> p g hh dd", dd=D),
                        bc32(msg[:, :, H:H + HEADS]))
                    for wi in range(2):
                        w = gidx * 2 + wi
                        # scatter: node-major [dst, 256 agg | 8 denom]; each
                        # chunk covers a 256-dst window, scattered in two
                        # 128-wide halves
                        pw = psSC.tile([128, 264], F32, tag="sc")
                        for kc in range(KCH):
                            nc.tensor.matmul(pw[:], ohg[:, kc, wi, :], msg[:, kc, :],
                                             start=(kc == 0), stop=(kc == KCH - 1))
                        # rec = 1/denom  [128 dst, 8] bf16
                        # +1e-30: degree-0 dst nodes have sum 0; keep 0*recip = 0
                        recf = sp.tile([128, HEADS], F32, tag="recf")
                        nc.vector.tensor_scalar_add(recf[:], pw[:, H:H + HEADS], 1e-30)
                        rec = sp.tile([128, HEADS], BF16, tag="rec")
                        with nc.allow_low_precision(reason="softmax recip to bf16"):
                            nc.vector.reciprocal(rec[:], recf[:])
                        # normalized node-major agg, bf16
                        an = sp.tile([128, H], BF16, tag="an")
                        nc.vector.tensor_mul(
                            an[:].rearrange("p (hh dd) -> p hh dd", dd=D),
                            pw[:, 0:H].rearrange("p (hh dd) -> p hh dd", dd=D),
                            bc32(rec[:]))
                        # transpose to feature-major for m_rel
                        anP = psAG.tile([128, 2, 128], BF16, tag="ag")
                        for ft in range(2):
                            nc.tensor.transpose(
                                anP[:, ft, :], an[:, ft * 128:(ft + 1) * 128],
                                identb[:])
                        anT = sp.tile([128, 2, 128], BF16, tag="anT")
                        nc.any.tensor_copy(anT[:], anP[:])
                        # m_rel block-diag transform (feature-major)
                        aggM = psAG.tile([128, 2, 128], F32, tag="ag")
                        for kt in range(2):
                            nc.tensor.matmul(aggM[:, kt, :], mblk_t[:, kt, :],
                                             anT[:, kt, :], start=True, stop=True)
                        dst_ap = aggF[:, AGG_SLOT[dt], :, w, :]
                        if first_for_dst:
                            nc.any.tensor_copy(dst_ap, aggM[:])
                        else:
                            nc.vector.tensor_add(dst_ap, dst_ap, aggM[:])
                nc.leave_named_scope(f"l{l}_r{r}", _sid, False)
                return last_gather

            def phase2(l, t):
                """gelu + Wa + gated skip + residual + LayerNorm + relu for
                dst type t, batched over all windows."""
                _sid, _ = nc.enter_named_scope(f"l{l}_p2t{t}", False)
                wa_t = load_w(wa_d[l, t])
                ba_t = load_bias(bias_d["ba_b"][l, t]) if ub["ba"] else None
                s1 = sp.tile([128, NT], F32, tag="s1")
                s2 = sp.tile([128, NT], F32, tag="s2")
                # gelus batched back-to-back (few ACT function-set swaps)
                NH = NT // 4
                geluF = [gp.tile([128, NH, 2, 128], BF16, tag=f"geluF{i}",
                                 name=f"geluF{i}") for i in range(4)]
                for w in range(NT):
                    nc.scalar.activation(geluF[w // NH][:, w % NH, :, :],
                                         aggF[:, AGG_SLOT[t], :, w, :], AF.Gelu)
                for w in range(NT):
                    po = psPO.tile([128, H], F32, tag="po")
                    for kt in range(2):
                        nc.tensor.matmul(po[:], geluF[w // NH][:, w % NH, kt, :],
                                         wa_t[:, kt, :],
                                         start=(kt == 0), stop=(kt == 1))
                    if ba_t is not None:
                        nc.vector.tensor_add(po[:], po[:], ba_t[:])
                    # h_pre = o + h (in place), s1 = row sums
                    nc.vector.scalar_tensor_tensor(
                        h[:, t, w, :], po[:], 1.0, h[:, t, w, :],
                        OP.mult, OP.add, accum_out=s1[:, w:w + 1])
                    sqs = sp.tile([128, H], F32, tag="sqs")
                    nc.scalar.activation(sqs[:], h[:, t, w, :], AF.Square,
                                         accum_out=s2[:, w:w + 1])
                # LayerNorm row stats
                mu = sp.tile([128, NT], F32, tag="mu")
                inv = sp.tile([128, NT], F32, tag="inv")
                nmi = sp.tile([128, NT], F32, tag="nmi")
                nc.vector.tensor_scalar_mul(mu[:], s1[:], 1.0 / H)
                nc.vector.tensor_scalar_mul(inv[:], s2[:], 1.0 / H)  # mean sq
                musq = sp.tile([128, NT], F32, tag="musq")
                nc.vector.tensor_mul(musq[:], mu[:], mu[:])
                nc.vector.scalar_tensor_tensor(
                    inv[:], inv[:], float(eps_eff[l][t]), musq[:],
                    OP.add, OP.subtract)              # var + eps
                nc.scalar.activation(inv[:], inv[:], AF.Sqrt)
                nc.vector.reciprocal(inv[:], inv[:])
                nc.vector.scalar_tensor_tensor(
                    nmi[:], mu[:], -1.0, inv[:], OP.mult, OP.mult)
                if ub["lng"] or ub["lnb"]:
                    lng_t = load_bias(bias_d["lng_b"][l, t])
                    lnb_t = load_bias(bias_d["lnb_b"][l, t])
                    for w in range(NT):
                        nc.scalar.activation(
                            h[:, t, w, :], h[:, t, w, :], AF.Identity,
                            bias=nmi[:, w:w + 1], scale=inv[:, w:w + 1])
                        nc.vector.tensor_mul(h[:, t, w, :], h[:, t, w, :], lng_t[:])
                        nc.vector.tensor_add(h[:, t, w, :], h[:, t, w, :], lnb_t[:])
                        nc.scalar.activation(h[:, t, w, :], h[:, t, w, :], AF.Relu)
                else:
                    for w in range(NT):
                        nc.scalar.activation(
                            h[:, t, w, :], h[:, t, w, :], AF.Relu,
                            bias=nmi[:, w:w + 1], scale=inv[:, w:w + 1])
                nc.leave_named_scope(f"l{l}_p2t{t}", _sid, False)

            def outproj_type(t):
                bo = load_bias(bias_d["bout_b"]) if ub["bout"] else None
                with nc.named_scope(f"outproj{t}"):
                    for nt in range(NT):
                        hTt = transpose_tile(h[:, t, nt, :], nt)
                        ps = psPO.tile([128, OUT], F32, tag="po")
                        for kt in range(2):
                            nc.tensor.matmul(ps[:], hTt[:, kt, :], wo_t[:, kt, :OUT],
                                             start=(kt == 0), stop=(kt == 1))
                        st_ = stg.tile([128, OUT], F32, tag="yout")
                        if bo is not None:
                            nc.vector.tensor_add(st_[:], ps[:], bo[:, :OUT])
                        else:
                            nc.any.tensor_copy(st_[:], ps[:])
                        nc.sync.dma_start(y_d[t, nt * 128:(nt + 1) * 128, :], st_[:])

            def finish_stage(l, t):
                """phase 2 for (l, t), then feed the next consumer: layer
                l+1's kv projections (types 0/1; the AllGather trigger is
                emitted separately) or the output projection."""
                phase2(l, t)
                if l < L - 1:
                    if t < 2:
                        with nc.named_scope(f"l{l + 1}_kv{t}"):
                            transpose_into_hTbig(t)
                            kvproj(l + 1, t)
                else:
                    outproj_type(t)

            # ================= schedule =================
            # layer 0 head, ordered for the earliest possible first edge
            # relation (r1: src t1 table + q' from dst-t0 h): build the t1
            # table first, then r1's q' dependencies; the t0 table (only
            # needed by r0, fourth relation) builds last.
            with nc.named_scope("l0_head"):
                with nc.named_scope("l0_kvfull1"):
                    fullN_kv_build(1)
                with nc.named_scope("inproj0"):
                    inproj_type(0)
                with nc.named_scope("l0_hT0"):
                    transpose_into_hTbig(0)
                with nc.named_scope("qproj1"):
                    qproj(0, 1)
                with nc.named_scope("inproj1"):
                    inproj_type(1)
                with nc.named_scope("l0_hT1"):
                    transpose_into_hTbig(1)
                with nc.named_scope("qproj2"):
                    qproj(0, 2)
                    qproj(0, 0)
                with nc.named_scope("inproj2"):
                    inproj_type(2)
                with nc.named_scope("qproj3"):
                    qproj(0, 3)
                with nc.named_scope("l0_kvfull0"):
                    fullN_kv_build(0)

            # layer 0 edges: r1 (t0 done early -> layer-1 kv for t0 staged),
            # then r2/r0 (t1), then r3 (t2). No Pool-blocking collectives in
            # this span, so gathers stream freely.
            edge_rel(0, 1, True)
            finish_stage(0, 0)          # phase2(t0) + kvproj(1, 0)
            edge_rel(0, 2, True)
            g_r0 = edge_rel(0, 0, False)
            finish_stage(0, 1)          # phase2(t1) + kvproj(1, 1)

            # layer-1 collectives + edges. Pool queue order: [l0 gathers]
            # AG1(t0) [r0' gathers] AG1(t1) [r2'/r1'/r3' gathers] -- every
            # gather after a collective also depends on it, so the blocking
            # collective costs the Pool queue nothing.
            cc10 = ag_trigger(1, 0, after=g_r0)
            # r3 (dst t2) is only needed by the very last tail relation, so
            # its gathers yield the Pool queue to the collectives
            edge_rel(0, 3, True, after_cc=cc10)
            finish_stage(0, 2)
            cc11 = ag_trigger(1, 1)
            with nc.named_scope("l1_qproj"):
                for r in EDGE_ORDERS[1]:
                    qproj(1, r)
            edge_rel(1, 0, True, after_cc=cc11)
            edge_rel(1, 2, False)
            finish_stage(1, 1)          # phase2(t1) + outproj(1)
            edge_rel(1, 1, True)
            finish_stage(1, 0)
            edge_rel(1, 3, True, vw_pool=True)
            finish_stage(1, 2)
    nc.compile()
    return nc


def kernel(**inputs):
    shared, per_core, meta = _preprocess(inputs)
    shapes = {k: list(v.shape) for k, v in {**shared, **per_core[0]}.items()}
    nc = bacc.Bacc("TRN2", target_bir_lowering=False, debug=False, num_devices=NC)
    nc = _build(nc, meta, shapes)
    in_maps = [{**shared, **per_core[c]} for c in range(NC)]
    res = run_bass_kernel_spmd(nc, in_maps, core_ids=list(range(NC)))
    y = np.concatenate([res.results[c]["y"][:, :NL, :] for c in range(NC)], axis=1)
    return y.astype(np.float32)


if __name__ == "__main__":
    import reference
    inputs = {k: np.asarray(v) for k, v in reference.setup_inputs().items()}
    out = kernel(**inputs)
    exp = np.asarray(reference.reference(**inputs))
    err = np.abs(out - exp).max() / np.abs(exp).max()
    print("Relative error:", err)


# revision 37
# speedup vs baseline: 1.8059x; 1.8059x over previous
"""HGT link predictor on 8 trn2 NeuronCores.

Sharding: nodes split 8 ways per type (2500/core, padded to 2560).
Params replicated. Edges partitioned by destination core, sorted by dst,
packed into 128-edge chunks within 128-dst-node windows.

v5 design (on top of v2):
- All matmul operands bf16 (PSUM accumulates fp32); a_rel folded into
  per-relation q projections; m_rel applied post-aggregation; scatter-add
  via one-hot matmuls with the softmax denominator riding as 8 extra
  columns (as in v2).
- Layer 0 needs NO collectives: x is replicated on every core, so each
  core computes the full-N layer-0 k/v table locally. The input
  projection for the kv path is done directly in feature-major form
  (hT = relu(Win^T @ x^T), two 128-wide matmuls per node tile -- no
  transposes), then k/v projections write the full 20480-row kv table
  straight to local DRAM. Only layer 1 performs the two kv AllGathers.
- Collectives block the issuing (gpsimd) engine queue until completion,
  and dma_gather lives on the same queue. The schedule orders Pool work
  so every gather that follows an AllGather in queue order also depends
  on it: [all layer-0 gathers] AG1(t0) [r0' gathers] AG1(t1) [r2'/r1'/
  r3' gathers]. kv projections for layer 1 are produced inside layer
  0's per-type finish stages so each AllGather's input is staged well
  before the queue reaches it.
- Deferred per-dst-type phase 2: relation processing only accumulates the
  normalized, m_rel-transformed aggregate (feature-major bf16). gelu +
  Wa + gated skip + residual + LayerNorm run as one batched pass per dst
  type, cutting ACT function-table swaps (exp vs gelu sets), and in the
  last layer each type flows straight into the output projection.
- Edges are packed into 128-edge chunks within 256-dst-node windows
  (84% slot utilization vs 59% at 128-node windows), shrinking gather
  rows and per-edge DVE work by ~30%; the one-hot scatter runs as two
  128-wide matmuls per chunk into the two PSUM window halves.
- Engine-flexible copies/relus use nc.any so the Tile scheduler routes
  them to whichever of DVE/ACT is idle.
"""
import math
import numpy as np

import concourse.bacc as bacc
import concourse.bass as bass
import concourse.mybir as mybir
import concourse.tile as tile
from concourse.bass_utils import run_bass_kernel_spmd
from concourse.library_config import mlp

F32 = mybir.dt.float32
BF16 = mybir.dt.bfloat16
I16 = mybir.dt.int16
AF = mybir.ActivationFunctionType
OP = mybir.AluOpType

T, R, L = 3, 4, 2
H, HEADS, D, FIN, OUT = 256, 8, 32, 128, 128
SRC_T = (0, 1, 1, 1)
DST_T = (1, 0, 1, 2)
LN_EPS = 1e-5
NC = 8
N = 20000
NL = N // NC          # 2500 real local nodes per type
NT = 20               # node tiles of 128
NLP = NT * 128        # 2560 padded local nodes
NWIN = NT             # dst windows of 128 local nodes
W2 = 256              # edge-packing window: 256 dst nodes (2 psum windows)
NWIN2 = NLP // W2     # 10 packing windows
KV_W = 2 * H          # 512: [k || v] columns of a kv-table row

# edge-relation processing order per layer (see module docstring)
EDGE_ORDERS = ((1, 2, 0, 3), (0, 2, 1, 3))


def _block_diag(a):
    """a: [HEADS, D, D] -> [H, H] block diagonal."""
    out = np.zeros((H, H), np.float32)
    for h in range(HEADS):
        out[h * D:(h + 1) * D, h * D:(h + 1) * D] = a[h]
    return out


def _wrap_idx(idx):
    """idx [M] -> [128, M//16] int16 wrapped in 16 partitions, replicated."""
    m = idx.shape[0]
    assert m % 16 == 0
    w = np.zeros((128, m // 16), np.int16)
    w[:16] = idx.astype(np.int16).reshape(m // 16, 16).T
    for rep in range(1, 8):
        w[16 * rep:16 * rep + 16] = w[:16]
    return w


def _preprocess(inputs):
    x = np.asarray(inputs["x"], np.float32)
    edge_index = np.asarray(inputs["edge_index"])
    Win = np.asarray(inputs["Win"], np.float32)
    b_in = np.asarray(inputs["b_in"], np.float32)
    Wk = np.asarray(inputs["Wk"], np.float32); bk = np.asarray(inputs["bk"], np.float32)
    Wq = np.asarray(inputs["Wq"], np.float32); bq = np.asarray(inputs["bq"], np.float32)
    Wv = np.asarray(inputs["Wv"], np.float32); bv = np.asarray(inputs["bv"], np.float32)
    Wa = np.asarray(inputs["Wa"], np.float32); ba = np.asarray(inputs["ba"], np.float32)
    skip = np.asarray(inputs["skip"], np.float32)
    a_rel = np.asarray(inputs["a_rel"], np.float32)
    m_rel = np.asarray(inputs["m_rel"], np.float32)
    p_rel = np.asarray(inputs["p_rel"], np.float32)
    ln_g = np.asarray(inputs["ln_g"], np.float32)
    ln_b = np.asarray(inputs["ln_b"], np.float32)
    Wout = np.asarray(inputs["Wout"], np.float32)
    bout = np.asarray(inputs["bout"], np.float32)

    meta = {}
    inv_sqrt_d = 1.0 / math.sqrt(D)
    # fold a_rel (scaled) into dst-side q projections per relation
    wq_eff = np.zeros((L, R, H, H), np.float32)
    bq_eff = np.zeros((L, R, H), np.float32)
    # block-diag m_rel chunks for post-aggregation transform (lhsT layout)
    m_blk = np.zeros((L, R, 2, 128, 128), np.float32)
    for l in range(L):
        for r in range(R):
            dt = DST_T[r]
            at = _block_diag(np.transpose(a_rel[l, r], (0, 2, 1))
                             * (p_rel[l, r] * inv_sqrt_d)[:, None, None])
            wq_eff[l, r] = Wq[l, dt] @ at
            bq_eff[l, r] = bq[l, dt] @ at
            mb = _block_diag(m_rel[l, r])
            m_blk[l, r, 0] = mb[0:128, 0:128]
            m_blk[l, r, 1] = mb[128:256, 128:256]
    beta = 1.0 / (1.0 + np.exp(-skip))          # [L, T]
    g = beta / (2.0 - beta)
    wa_eff = Wa * g[:, :, None, None]
    ba_eff = ba * g[:, :, None]
    meta["eps_eff"] = (LN_EPS / (2.0 - beta) ** 2).tolist()

    meta["use_bias"] = dict(
        bin_=bool(np.any(b_in)), bq=bool(np.any(bq_eff)),
        bkv=bool(np.any(bk[:, :2])) or bool(np.any(bv[:, :2])),
        ba=bool(np.any(ba_eff)), bout=bool(np.any(bout)),
        lng=not np.allclose(ln_g, 1.0), lnb=bool(np.any(ln_b)),
    )

    def bcast(v):
        # [..., F] -> [..., 128, F]: per-feature vectors replicated across partitions
        return np.ascontiguousarray(
            np.broadcast_to(v[..., None, :], v.shape[:-1] + (128, v.shape[-1])))

    # edge partitioning ---------------------------------------------------
    # edges are packed densely into 128-edge chunks within 256-dst-node
    # windows (better chunk utilization than 128-node windows); the one-hot
    # scatter runs as two 128-wide matmuls per chunk.
    win_edges = [[] for _ in range(NC)]   # [c][r][w2] -> (src_rows, dst_loc)
    kch_need = 1
    for c in range(NC):
        rel = []
        for r in range(R):
            src = edge_index[r, 0].astype(np.int64)
            dst = edge_index[r, 1].astype(np.int64)
            sel = (dst // NL) == c
            s, d = src[sel], dst[sel] - c * NL
            o = np.argsort(d, kind="stable")
            s, d = s[o], d[o]
            wins = []
            for w in range(NWIN2):
                m = (d // W2) == w
                sw, dw = s[m], d[m]
                kch_need = max(kch_need, (len(sw) + 127) // 128)
                wins.append((sw, dw))
            rel.append(wins)
        win_edges[c] = rel
    KCH = kch_need
    meta["KCH"] = KCH
    NCHUNK = NWIN2 * KCH
    NIDX_R = NCHUNK * 128

    per_core = []
    for c in range(NC):
        oh = np.zeros((R, NCHUNK, 128, W2), np.float32)
        kv_idx = np.zeros((R, NIDX_R), np.int64)
        qi_idx = np.zeros((R, NIDX_R), np.int64)
        for r in range(R):
            for w in range(NWIN2):
                sw, dw = win_edges[c][r][w]
                ne = len(sw)
                base = w * KCH * 128
                # src node n (global) -> kv-table row (n//NL)*NLP + n%NL
                kv_idx[r, base:base + ne] = (sw // NL) * NLP + (sw % NL)
                qi_idx[r, base:base + ne] = dw
                ch = base // 128 + np.arange(ne) // 128
                oh[r, ch, np.arange(ne) % 128, dw - w * W2] = 1.0
        # partition-major one-hot halves: [R, 128(edge), NCHUNK, 2, 128(col)]
        oh_pm = np.ascontiguousarray(
            oh.transpose(0, 2, 1, 3).reshape(R, 128, NCHUNK, 2, 128))
        xc = np.zeros((T, 128, NLP), np.float32)
        xc[:, :, :NL] = x[:, c * NL:(c + 1) * NL, :].transpose(0, 2, 1)
        per_core.append(dict(
            xT_h=_bf(xc),
            oh=_bf(oh_pm),
            kv_idx=np.stack([_wrap_idx(kv_idx[r]) for r in range(R)]),
            qi_idx=np.stack([_wrap_idx(qi_idx[r]) for r in range(R)]),
        ))

    # full-N x for source types 0/1, feature-major, padded per core shard
    # (row of global node n of type t in the kv table: (n//NL)*NLP + n%NL)
    xT_full = np.zeros((2, 128, NC * NLP), np.float32)
    for c in range(NC):
        xT_full[:, :, c * NLP:c * NLP + NL] = \
            x[:2, c * NL:(c + 1) * NL, :].transpose(0, 2, 1)

    shared = dict(
        xT_full=_bf(xT_full),
        win=_bf(Win),                                     # [3,128,256]
        wk=_bf(Wk[:, :2]), wv=_bf(Wv[:, :2]),             # [L,2,256,256]
        wq=_bf(wq_eff), wa=_bf(wa_eff),
        m_blk=_bf(m_blk),
        wout=_bf(Wout),
        ident=np.eye(128, dtype=np.float32),
        identb=_bf(np.eye(128, dtype=np.float32)),
        bin_b=bcast(b_in), bq_b=bcast(bq_eff),
        bk_b=bcast(bk[:, :2]), bv_b=bcast(bv[:, :2]),
        ba_b=bcast(ba_eff), bout_b=bcast(bout),
        lng_b=bcast(ln_g), lnb_b=bcast(ln_b),
    )
    return shared, per_core, meta


def _bf(a):
    import ml_dtypes
    return np.ascontiguousarray(a).astype(ml_dtypes.bfloat16)


def NIDX_R16(KCH):
    return NWIN2 * KCH * 128 // 16


def _build(nc, meta, shapes):
    KCH = meta["KCH"]
    NCHUNK = NWIN2 * KCH
    GC = KCH                             # chunks per gather group (1 window)
    NGRP = NWIN2
    ub = meta["use_bias"]
    eps_eff = meta["eps_eff"]

    def din(name, dt_):
        return nc.dram_tensor(name, shapes[name], dt_, kind="ExternalInput").ap()

    xT_h = din("xT_h", BF16); oh_d = din("oh", BF16)
    xT_full = din("xT_full", BF16)
    kv_idx_d = din("kv_idx", I16); qi_idx_d = din("qi_idx", I16)
    win_d = din("win", BF16)
    wk_d = din("wk", BF16); wv_d = din("wv", BF16)
    wq_d = din("wq", BF16); wa_d = din("wa", BF16)
    m_blk_d = din("m_blk", BF16)
    wout_d = din("wout", BF16)
    ident_d = din("ident", F32); identb_d = din("identb", BF16)
    bias_d = {k: din(k, F32) for k in
              ("bin_b", "bq_b", "bk_b", "bv_b", "ba_b", "bout_b", "lng_b", "lnb_b")}
    y_d = nc.dram_tensor("y", [T, NLP, OUT], F32, kind="ExternalOutput").ap()

    def bc32(ap2d):
        """[..., k] AP -> [..., k, 32] stride-0 broadcast AP."""
        return bass.AP(tensor=ap2d.tensor, offset=ap2d.offset,
                       ap=list(ap2d.ap) + [[0, D]])

    with tile.TileContext(nc) as tc:
        with (
            tc.tile_pool(name="persist", bufs=1) as pp,
            tc.tile_pool(name="wpool", bufs=6) as wp,
            tc.tile_pool(name="wsmall", bufs=2) as ws,
            tc.tile_pool(name="stage", bufs=2) as stg,
            tc.tile_pool(name="gathk", bufs=4) as gkp,
            tc.tile_pool(name="gath", bufs=3) as gep,
            tc.tile_pool(name="edge", bufs=2) as ep,
            tc.tile_pool(name="small", bufs=3) as sp,
            tc.tile_pool(name="gelu", bufs=1) as gp,
            tc.tile_pool(name="idx", bufs=2) as ip,
            tc.tile_pool(name="psSC", bufs=3, space="PSUM") as psSC,
            tc.tile_pool(name="psAG", bufs=2, space="PSUM") as psAG,
            tc.tile_pool(name="psPO", bufs=3, space="PSUM") as psPO,
            tc.tile_pool(name="dram", bufs=1, space="DRAM") as dp,
        ):
            nc.gpsimd.load_library(mlp)

            ident = pp.tile([128, 128], F32, tag="ident")
            nc.sync.dma_start(ident[:], ident_d)
            identb = pp.tile([128, 128], BF16, tag="identb")
            nc.sync.dma_start(identb[:], identb_d)
            h = pp.tile([128, T, NT, H], F32, tag="h")
            # feature-major normalized+m_rel aggregate, phase-2 input.
            # Slot 0 holds dst type 1 (accumulated across its two relations,
            # long-lived); types 0 and 2 take turns in slot 1.
            aggF = pp.tile([128, 2, 2, NT, 128], BF16, tag="aggF")
            AGG_SLOT = {1: 0, 0: 1, 2: 1}
            # feature-major h of types 0/1 (kv + q projection input)
            hTbig = pp.tile([128, 2, 2, NT, 128], BF16, tag="hTbig")
            wo_t = pp.tile([128, 2, OUT], BF16, tag="wo")
            nc.sync.dma_start(wo_t[:], wout_d.rearrange("(kt kp) m -> kp kt m", kp=128))

            # layer 0: full-N kv tables built locally (x is replicated);
            # layer 1: per-core shard staged to kv_loc then AllGathered.
            kv_loc = [None, [dp.tile([NLP, KV_W], BF16, name=f"kv_loc1{t}")
                             for t in range(2)]]
            kv_full = [[dp.tile([NC * NLP, KV_W], BF16, name=f"kv_full0{t}")
                        for t in range(2)],
                       [dp.tile([NC * NLP, KV_W], BF16, addr_space="Shared",
                                name=f"kv_full1{t}")
                        for t in range(2)]]
            q_dram = [dp.tile([R, NLP, H], BF16, name=f"q_dram{l}")
                      for l in range(L)]

            def load_w(src_ap):
                """[256, M] bf16 dram -> [128, 2, M] sbuf tile."""
                m = src_ap.shape[-1]
                t_ = wp.tile([128, 2, m], BF16, tag="w")
                nc.sync.dma_start(t_[:], src_ap.rearrange("(kt kp) m -> kp kt m", kp=128))
                return t_

            def load_bias(src_ap):
                t_ = wp.tile([128, H], F32, tag="bias")
                nc.sync.dma_start(t_[:], src_ap)
                return t_

            # ---- input projection: h[t] = relu(xT^T @ Win + b) ----
            def inproj_type(t):
                w_in = ws.tile([128, H], BF16, tag="win", name="w_in")
                nc.sync.dma_start(w_in[:], win_d[t])
                bt = load_bias(bias_d["bin_b"][t]) if ub["bin_"] else None
                for nt in range(NT):
                    xt = ws.tile([128, 128], BF16, tag="xt", name="xt")
                    nc.sync.dma_start(xt[:], xT_h[t, :, nt * 128:(nt + 1) * 128])
                    ps = psPO.tile([128, H], F32, tag="po", name="ps_in")
                    nc.tensor.matmul(ps[:], xt[:], w_in[:], start=True, stop=True)
                    if bt is not None:
                        nc.vector.tensor_add(ps[:], ps[:], bt[:])
                    nc.any.tensor_scalar(h[:, t, nt, :], ps[:], 0.0, None, OP.max)

            def transpose_tile(src2, nt_label):
                """h tile [128, 256] f32 -> hT [128, 2, 128] bf16 (feature-major)."""
                hTt = ws.tile([128, 2, 128], BF16, tag="hTt")
                for ft in range(2):
                    tp = psSC.tile([128, 128], F32, tag="sc")
                    nc.tensor.transpose(tp[:], src2[:, ft * 128:(ft + 1) * 128], ident[:])
                    nc.any.tensor_copy(hTt[:, ft, :], tp[:])
                return hTt

            def transpose_into_hTbig(t):
                """Refresh hTbig[t] from the current h[t]."""
                for nt in range(NT):
                    for ft in range(2):
                        tp = psSC.tile([128, 128], F32, tag="sc", name="tp")
                        nc.tensor.transpose(
                            tp[:], h[:, t, nt, ft * 128:(ft + 1) * 128], ident[:])
                        nc.any.tensor_copy(hTbig[:, t, ft, nt, :], tp[:])

            def fullN_kv_build(t):
                """Layer-0 kv table for src type t over ALL N nodes, built
                locally from the replicated x. The input projection runs
                directly in feature-major form (hT = relu(Win^T @ xT)), so
                no transposes are needed; k/v write straight to local DRAM."""
                w_in = ws.tile([128, H], BF16, tag="win", name="w_in0")
                nc.sync.dma_start(w_in[:], win_d[t])
                wkt = load_w(wk_d[0, t]); wvt = load_w(wv_d[0, t])
                GSTG = 5
                for grp in range(NC * NT // GSTG):
                    kvstg = stg.tile([128, GSTG, KV_W], BF16, tag="kvstg",
                                     name="kvstgF")
                    xt = ws.tile([128, GSTG, 128], BF16, tag="xt", name="xtF")
                    nc.sync.dma_start(
                        xt[:], xT_full[t, :, grp * GSTG * 128:(grp + 1) * GSTG * 128]
                        .rearrange("p (gi n) -> p gi n", gi=GSTG))
                    for gi in range(GSTG):
                        g = grp * GSTG + gi
                        ps = psAG.tile([128, 2, 128], F32, tag="ag", name="ps_hT")
                        for half in range(2):
                            nc.tensor.matmul(
                                ps[:, half, :],
                                w_in[:, half * 128:(half + 1) * 128], xt[:, gi, :],
                                start=True, stop=True)
                        hTn = ws.tile([128, 2, 128], BF16, tag="hTn", name="hTn")
                        nc.any.tensor_scalar(hTn[:], ps[:], 0.0, None, OP.max)
                        for (wt, col) in ((wkt, 0), (wvt, H)):
                            po = psPO.tile([128, H], F32, tag="po", name="ps_kvF")
                            for kt in range(2):
                                nc.tensor.matmul(po[:], hTn[:, kt, :],
                                                 wt[:, kt, :],
                                                 start=(kt == 0), stop=(kt == 1))
                            nc.any.tensor_copy(kvstg[:, gi, col:col + H], po[:])
                    r0_ = grp * GSTG * 128
                    r1_ = r0_ + GSTG * 128
                    nc.sync.dma_start(
                        kv_full[0][t][r0_:r1_, :].rearrange(
                            "(nt kp) m -> kp nt m", kp=128),
                        kvstg[:])

            def kvproj(l, t):
                """kv projections for layer l source type t from hTbig[t],
                staged to kv_loc (the AllGather trigger is emitted separately
                so it lands in the right spot in the Pool queue order)."""
                wkt = load_w(wk_d[l, t]); wvt = load_w(wv_d[l, t])
                bkt = load_bias(bias_d["bk_b"][l, t]) if ub["bkv"] else None
                bvt = load_bias(bias_d["bv_b"][l, t]) if ub["bkv"] else None
                QTR = NT // 4
                for quarter in range(4):
                    kvstg = stg.tile([128, QTR, KV_W], BF16, tag="kvstg",
                                     name="kvstg")
                    for nti in range(QTR):
                        nt = quarter * QTR + nti
                        for ci, (wt, bt, col) in enumerate(
                                ((wkt, bkt, 0), (wvt, bvt, H))):
                            ps = psPO.tile([128, H], F32, tag="po", name="ps_kv")
                            for kt in range(2):
                                nc.tensor.matmul(ps[:], hTbig[:, t, kt, nt, :],
                                                 wt[:, kt, :],
                                                 start=(kt == 0), stop=(kt == 1))
                            dst_ = kvstg[:, nti, col:col + H]
                            if bt is not None:
                                nc.vector.tensor_add(dst_, ps[:], bt[:])
                            else:
                                nc.any.tensor_copy(dst_, ps[:])
                    r0_ = quarter * QTR * 128
                    r1_ = r0_ + QTR * 128
                    nc.sync.dma_start(
                        kv_loc[l][t][r0_:r1_, :].rearrange(
                            "(nt kp) m -> kp nt m", kp=128),
                        kvstg[:])

            def ag_trigger(l, t, after=None):
                """AllGather kv_loc -> kv_full. Blocks the Pool queue until
                the collective completes, so call sites place this only
                where all following Pool work depends on it anyway.
                `after` pins the trigger behind a gather instruction so the
                scheduler cannot insert the collective mid-stream and hijack
                gathers the collective's own input depends on."""
                with nc.named_scope(f"l{l}_ag{t}"):
                    cc = nc.gpsimd.collective_compute(
                        "AllGather", OP.bypass,
                        replica_groups=[list(range(NC))],
                        ins=[kv_loc[l][t][:].opt()],
                        outs=[kv_full[l][t][:].opt()],
                    )
                    if after is not None:
                        bass._add_dep_helper(
                            cc.ins, after.ins, sync=False,
                            reason="AG waits for the gathers feeding its input")
                return cc

            def qproj(l, r):
                """q' projection for relation r (a_rel folded), to q_dram."""
                dt_ = DST_T[r]
                wqr = load_w(wq_d[l, r])
                bqr = load_bias(bias_d["bq_b"][l, r]) if ub["bq"] else None
                for quarter in range(4):
                    qstg = stg.tile([128, NT // 4, H], BF16, tag="qstg",
                                    name=f"qstg{r}")
                    for nti in range(NT // 4):
                        nt = quarter * (NT // 4) + nti
                        if dt_ < 2:
                            hTt2 = hTbig[:, dt_, :, nt, :]
                        else:
                            hTt = transpose_tile(h[:, dt_, nt, :], nt)
                            hTt2 = hTt[:]
                        ps = psPO.tile([128, H], F32, tag="po", name="ps_q")
                        for kt in range(2):
                            nc.tensor.matmul(ps[:], hTt2[:, kt, :],
                                             wqr[:, kt, :],
                                             start=(kt == 0), stop=(kt == 1))
                        if bqr is not None:
                            nc.vector.tensor_add(qstg[:, nti, :], ps[:], bqr[:])
                        else:
                            nc.any.tensor_copy(qstg[:, nti, :], ps[:])
                    r0_ = quarter * (NT // 4) * 128
                    r1_ = r0_ + (NT // 4) * 128
                    nc.sync.dma_start(
                        q_dram[l][r, r0_:r1_, :].rearrange(
                            "(nt kp) m -> kp nt m", kp=128),
                        qstg[:])

            def edge_rel(l, r, first_for_dst, after_cc=None, vw_pool=False):
                """Edge phase for relation r: gather kv/q', logits, softmax
                numerators, one-hot scatter, normalize, m_rel; accumulate
                feature-major result into aggF[dst]."""
                _sid, _ = nc.enter_named_scope(f"l{l}_r{r}", False)
                dt = DST_T[r]
                st = SRC_T[r]
                mblk_t = ws.tile([128, 2, 128], BF16, tag="mblk")
                nc.sync.dma_start(mblk_t[:], m_blk_d[l, r].rearrange("kt p m -> p kt m"))
                kvi = ip.tile([128, NIDX_R16(KCH)], I16, tag="kvi")
                qii = ip.tile([128, NIDX_R16(KCH)], I16, tag="qii")
                nc.sync.dma_start(kvi[:], kv_idx_d[r])
                nc.sync.dma_start(qii[:], qi_idx_d[r])
                for gidx in range(NGRP):
                    ni = GC * 128
                    kvg = gkp.tile([128, GC, KV_W], BF16, tag="kvg")
                    qig = gep.tile([128, GC, H], BF16, tag="qig")
                    gi_ = nc.gpsimd.dma_gather(
                        kvg[:], kv_full[l][st][:],
                        kvi[:, gidx * (ni // 16):(gidx + 1) * (ni // 16)],
                        ni, ni, KV_W)
                    if after_cc is not None and gidx == 0:
                        bass._add_dep_helper(
                            gi_.ins, after_cc.ins, sync=False,
                            reason="keep Pool queue clear ahead of the AG")
                    if gidx == NGRP - 1:
                        last_gather = gi_
                    nc.gpsimd.dma_gather(
                        qig[:], q_dram[l][r],
                        qii[:, gidx * (ni // 16):(gidx + 1) * (ni // 16)],
                        ni, ni, H)
                    ohg = ep.tile([128, GC, 2, 128], BF16, tag="ohg")
                    nc.sync.dma_start(ohg[:], oh_d[r, :, gidx * GC:(gidx + 1) * GC, :, :])
                    msg = ep.tile([128, GC, H + HEADS], BF16, tag="msg")
                    lg = sp.tile([128, GC, HEADS], F32, tag="lg")
                    # q*k product staged in msg[:, :, 0:H]; overwritten by
                    # the weighted-v below after the reduce consumes it.
                    # Runs on gpsimd: it depends only on the two gathers
                    # (same engine, just upstream), Pool sits idle between
                    # gathers, and DVE is the saturated engine.
                    nc.gpsimd.tensor_mul(msg[:, :, 0:H], qig[:], kvg[:, :, 0:H])
                    nc.vector.tensor_reduce(
                        lg[:], msg[:, :, 0:H].rearrange("p g (hh dd) -> p g hh dd", dd=D),
                        mybir.AxisListType.X, OP.add)
                    nc.scalar.activation(msg[:, :, H:H + HEADS], lg[:], AF.Exp)
                    # the last relation in the schedule may run this on
                    # gpsimd too (Pool has nothing queued after it)
                    vw_eng = nc.gpsimd if vw_pool else nc.vector
                    vw_eng.tensor_mul(
                        msg[:, :, 0:H].rearrange("p g (hh dd) -> p g hh dd", dd=D),
                        kvg[:, :, H:2 * H].rearrange("p g (hh dd) -> p g hh dd", dd=D),
                        bc32(msg[:, :, H:H + HEADS]))
                    for wi in range(2):
                        w = gidx * 2 + wi
                        # scatter: node-major [dst, 256 agg | 8 denom]; each
                        # chunk covers a 256-dst window, scattered in two
                        # 128-wide halves
                        pw = psSC.tile([128, 264], F32, tag="sc")
                        for kc in range(KCH):
                            nc.tensor.matmul(pw[:], ohg[:, kc, wi, :], msg[:, kc, :],
                                             start=(kc == 0), stop=(kc == KCH - 1))
                        # rec = 1/denom  [128 dst, 8] bf16
                        # +1e-30: degree-0 dst nodes have sum 0; keep 0*recip = 0
                        recf = sp.tile([128, HEADS], F32, tag="recf")
                        nc.vector.tensor_scalar_add(recf[:], pw[:, H:H + HEADS], 1e-30)
                        rec = sp.tile([128, HEADS], BF16, tag="rec")
                        with nc.allow_low_precision(reason="softmax recip to bf16"):
                            nc.vector.reciprocal(rec[:], recf[:])
                        # normalized node-major agg, bf16
                        an = sp.tile([128, H], BF16, tag="an")
                        nc.vector.tensor_mul(
                            an[:].rearrange("p (hh dd) -> p hh dd", dd=D),
                            pw[:, 0:H].rearrange("p (hh dd) -> p hh dd", dd=D),
                            bc32(rec[:]))
                        # transpose to feature-major for m_rel
                        anP = psAG.tile([128, 2, 128], BF16, tag="ag")
                        for ft in range(2):
                            nc.tensor.transpose(
                                anP[:, ft, :], an[:, ft * 128:(ft + 1) * 128],
                                identb[:])
                        anT = sp.tile([128, 2, 128], BF16, tag="anT")
                        nc.any.tensor_copy(anT[:], anP[:])
                        # m_rel block-diag transform (feature-major)
                        aggM = psAG.tile([128, 2, 128], F32, tag="ag")
                        for kt in range(2):
                            nc.tensor.matmul(aggM[:, kt, :], mblk_t[:, kt, :],
                                             anT[:, kt, :], start=True, stop=True)
                        dst_ap = aggF[:, AGG_SLOT[dt], :, w, :]
                        if first_for_dst:
                            nc.any.tensor_copy(dst_ap, aggM[:])
                        else:
                            nc.vector.tensor_add(dst_ap, dst_ap, aggM[:])
                nc.leave_named_scope(f"l{l}_r{r}", _sid, False)
                return last_gather

            def phase2(l, t):
                """gelu + Wa + gated skip + residual + LayerNorm + relu for
                dst type t, batched over all windows."""
                _sid, _ = nc.enter_named_scope(f"l{l}_p2t{t}", False)
                wa_t = load_w(wa_d[l, t])
                ba_t = load_bias(bias_d["ba_b"][l, t]) if ub["ba"] else None
                s1 = sp.tile([128, NT], F32, tag="s1")
                s2 = sp.tile([128, NT], F32, tag="s2")
                # gelus batched back-to-back (few ACT function-set swaps)
                NH = NT // 4
                geluF = [gp.tile([128, NH, 2, 128], BF16, tag=f"geluF{i}",
                                 name=f"geluF{i}") for i in range(4)]
                for w in range(NT):
                    nc.scalar.activation(geluF[w // NH][:, w % NH, :, :],
                                         aggF[:, AGG_SLOT[t], :, w, :], AF.Gelu)
                for w in range(NT):
                    po = psPO.tile([128, H], F32, tag="po")
                    for kt in range(2):
                        nc.tensor.matmul(po[:], geluF[w // NH][:, w % NH, kt, :],
                                         wa_t[:, kt, :],
                                         start=(kt == 0), stop=(kt == 1))
                    if ba_t is not None:
                        nc.vector.tensor_add(po[:], po[:], ba_t[:])
                    # h_pre = o + h (in place), s1 = row sums
                    nc.vector.scalar_tensor_tensor(
                        h[:, t, w, :], po[:], 1.0, h[:, t, w, :],
                        OP.mult, OP.add, accum_out=s1[:, w:w + 1])
                    sqs = sp.tile([128, H], F32, tag="sqs")
                    nc.scalar.activation(sqs[:], h[:, t, w, :], AF.Square,
                                         accum_out=s2[:, w:w + 1])
                # LayerNorm row stats
                mu = sp.tile([128, NT], F32, tag="mu")
                inv = sp.tile([128, NT], F32, tag="inv")
                nmi = sp.tile([128, NT], F32, tag="nmi")
                nc.vector.tensor_scalar_mul(mu[:], s1[:], 1.0 / H)
                nc.vector.tensor_scalar_mul(inv[:], s2[:], 1.0 / H)  # mean sq
                musq = sp.tile([128, NT], F32, tag="musq")
                nc.vector.tensor_mul(musq[:], mu[:], mu[:])
                nc.vector.scalar_tensor_tensor(
                    inv[:], inv[:], float(eps_eff[l][t]), musq[:],
                    OP.add, OP.subtract)              # var + eps
                nc.scalar.activation(inv[:], inv[:], AF.Sqrt)
                nc.vector.reciprocal(inv[:], inv[:])
                nc.vector.scalar_tensor_tensor(
                    nmi[:], mu[:], -1.0, inv[:], OP.mult, OP.mult)
                if ub["lng"] or ub["lnb"]:
                    lng_t = load_bias(bias_d["lng_b"][l, t])
                    lnb_t = load_bias(bias_d["lnb_b"][l, t])
                    for w in range(NT):
                        nc.scalar.activation(
                            h[:, t, w, :], h[:, t, w, :], AF.Identity,
                            bias=nmi[:, w:w + 1], scale=inv[:, w:w + 1])
                        nc.vector.tensor_mul(h[:, t, w, :], h[:, t, w, :], lng_t[:])
                        nc.vector.tensor_add(h[:, t, w, :], h[:, t, w, :], lnb_t[:])
                        nc.scalar.activation(h[:, t, w, :], h[:, t, w, :], AF.Relu)
                else:
                    for w in range(NT):
                        nc.scalar.activation(
                            h[:, t, w, :], h[:, t, w, :], AF.Relu,
                            bias=nmi[:, w:w + 1], scale=inv[:, w:w + 1])
                nc.leave_named_scope(f"l{l}_p2t{t}", _sid, False)

            def outproj_type(t):
                bo = load_bias(bias_d["bout_b"]) if ub["bout"] else None
                with nc.named_scope(f"outproj{t}"):
                    for nt in range(NT):
                        hTt = transpose_tile(h[:, t, nt, :], nt)
                        ps = psPO.tile([128, OUT], F32, tag="po")
                        for kt in range(2):
                            nc.tensor.matmul(ps[:], hTt[:, kt, :], wo_t[:, kt, :OUT],
                                             start=(kt == 0), stop=(kt == 1))
                        st_ = stg.tile([128, OUT], F32, tag="yout")
                        if bo is not None:
                            nc.vector.tensor_add(st_[:], ps[:], bo[:, :OUT])
                        else:
                            nc.any.tensor_copy(st_[:], ps[:])
                        nc.sync.dma_start(y_d[t, nt * 128:(nt + 1) * 128, :], st_[:])

            def finish_stage(l, t):
                """phase 2 for (l, t), then feed the next consumer: layer
                l+1's kv projections (types 0/1; the AllGather trigger is
                emitted separately) or the output projection."""
                phase2(l, t)
                if l < L - 1:
                    if t < 2:
                        with nc.named_scope(f"l{l + 1}_kv{t}"):
                            transpose_into_hTbig(t)
                            kvproj(l + 1, t)
                else:
                    outproj_type(t)

            # ================= schedule =================
            # layer 0 head, ordered for the earliest possible first edge
            # relation (r1: src t1 table + q' from dst-t0 h): build the t1
            # table first, then r1's q' dependencies; the t0 table (only
            # needed by r0, fourth relation) builds last.
            with nc.named_scope("l0_head"):
                with nc.named_scope("l0_kvfull1"):
                    fullN_kv_build(1)
                with nc.named_scope("inproj0"):
                    inproj_type(0)
                with nc.named_scope("l0_hT0"):
                    transpose_into_hTbig(0)
                with nc.named_scope("qproj1"):
                    qproj(0, 1)
                with nc.named_scope("inproj1"):
                    inproj_type(1)
                with nc.named_scope("l0_hT1"):
                    transpose_into_hTbig(1)
                with nc.named_scope("qproj2"):
                    qproj(0, 2)
                    qproj(0, 0)
                with nc.named_scope("inproj2"):
                    inproj_type(2)
                with nc.named_scope("qproj3"):
                    qproj(0, 3)
                with nc.named_scope("l0_kvfull0"):
                    fullN_kv_build(0)

            # layer 0 edges: r1 (t0 done early -> layer-1 kv for t0 staged),
            # then r2/r0 (t1), then r3 (t2). No Pool-blocking collectives in
            # this span, so gathers stream freely.
            edge_rel(0, 1, True)
            finish_stage(0, 0)          # phase2(t0) + kvproj(1, 0)
            edge_rel(0, 2, True)
            g_r0 = edge_rel(0, 0, False)
            finish_stage(0, 1)          # phase2(t1) + kvproj(1, 1)

            # layer-1 collectives + edges. Pool queue order: [l0 gathers]
            # AG1(t0) [r0' gathers] AG1(t1) [r2'/r1'/r3' gathers] -- every
            # gather after a collective also depends on it, so the blocking
            # collective costs the Pool queue nothing.
            cc10 = ag_trigger(1, 0, after=g_r0)
            # r3 (dst t2) is only needed by the very last tail relation, so
            # its gathers yield the Pool queue to the collectives
            edge_rel(0, 3, True, after_cc=cc10)
            finish_stage(0, 2)
            cc11 = ag_trigger(1, 1)
            with nc.named_scope("l1_qproj"):
                for r in EDGE_ORDERS[1]:
                    qproj(1, r)
            edge_rel(1, 0, True, after_cc=cc11)
            edge_rel(1, 2, False)
            finish_stage(1, 1)          # phase2(t1) + outproj(1)
            edge_rel(1, 1, True)
            finish_stage(1, 0)
            edge_rel(1, 3, True, vw_pool=True)
            finish_stage(1, 2)
    nc.compile()
    return nc


def kernel(**inputs):
    shared, per_core, meta = _preprocess(inputs)
    shapes = {k: list(v.shape) for k, v in {**shared, **per_core[0]}.items()}
    nc = bacc.Bacc("TRN2", target_bir_lowering=False, debug=False, num_devices=NC)
    nc = _build(nc, meta, shapes)
    in_maps = [{**shared, **per_core[c]} for c in range(NC)]
    res = run_bass_kernel_spmd(nc, in_maps, core_ids=list(range(NC)))
    y = np.concatenate([res.results[c]["y"][:, :NL, :] for c in range(NC)], axis=1)
    return y.astype(np.float32)


if __name__ == "__main__":
    import reference
    inputs = {k: np.asarray(v) for k, v in reference.setup_inputs().items()}
    out = kernel(**inputs)
    exp = np.asarray(reference.reference(**inputs))
    err = np.abs(out - exp).max() / np.abs(exp).max()
    print("Relative error:", err)
